# revision 53
# baseline (speedup 1.0000x reference)
"""Causal self-attention TRN2 Bass kernel (8 NeuronCores).

Sharding: core c handles batch b = c//4 and heads [4*(c%4), 4*(c%4)+4).
Each core computes its heads' QKV projection, causal attention, and the
partial output projection ctx_slice @ w_out_rows; the host sums the 4
partials per batch (exact, since the projection is linear over head
channels) and adds the constant bias terms.

Numerics: matmuls in float32r (TF32-like, ~13-bit mantissa, full PE rate
at N>=256); softmax logits in fp32 PSUM with exact row-max subtraction;
P and V in bf16 (linear error only).

Structure: softmax renormalization (flash-attention part-combine factors
f_i = exp(m_i - m)/s) runs as a tensor_scalar multiply over the SBUF P
tiles on the otherwise-idle GPSIMD engine (the only engine with no PSUM
access, so all PSUM->SBUF traffic stays on DVE/Act, balanced ~11:5 by
their measured copy rates). The attention loop is software-pipelined at
the emission level (engines execute in order, so emission order is the
schedule): phase B (transpose + P^T@V) of iteration n-1 is woven between
the score tiles of iteration n, the P^T@V matmul lags one k-tile behind
its PSUM->SBUF copy, and the diag-group combine is emitted one q-tile
behind the scores, keeping every engine's queue primed.

Note (hardware-verified): PE transpose-mode matmuls ignore the values of
the second operand (pure permute), so folding diag(f) into the transpose
does NOT work on silicon, although the cost-model simulator accepts it.
Custom-ISA ops (bass_isa InstISA class: TensorTensorReduce,
tensor_mask_reduce) and K=1 rank-1 bias matmuls crash at runtime on
this toolchain -- the runtime's ucode does not support them.
Weaving projection matmul groups between attention score tiles (sharing
the score PSUM ring) also corrupts results on silicon despite simulating
~14us faster and passing dependency tracking -- keep the projection
phase strictly before the attention loop.
"""
import math
import os

import numpy as np

import concourse.bacc as bacc
import concourse.bass as bass
import concourse.mybir as mybir
import concourse.tile as tile
from concourse.bass import ds, ts
from concourse.bass_utils import run_bass_kernel_spmd
from concourse.masks import make_identity

# problem shapes (hardcoded per contract)
B, T, C = 2, 2048, 1024
H, D = 16, 64
P = 128
CG = C // P            # 8 contraction tiles over channels
TT = T // P            # 16 token tiles of 128
NG = T // 512          # 4 q-groups of 512
HPAIRS = 2             # head-pairs per core (4 heads/core)
HC = 256               # head channels per core (4 heads * 64)
WLAST = [256, 256, 384, 512]   # ragged width of the diagonal k-tile per qt%4
PART = 512             # softmax part width (1 PSUM bank)
NEG = -1.0e30

F32 = mybir.dt.float32
F32R = mybir.dt.float32r
BF16 = mybir.dt.bfloat16
AX = mybir.AxisListType
OP = mybir.AluOpType
ACTF = mybir.ActivationFunctionType

_CACHE = {}
LAST_RESULT = None


def _build():
    ablate = set(os.environ.get("KERNEL_ABLATE", "").split(","))
    nc = bacc.Bacc("TRN2", target_bir_lowering=False, debug=False, num_devices=8)

    xT_d = nc.dram_tensor("xT", [C, T], F32R, kind="ExternalInput").ap()
    wq_d = nc.dram_tensor("wq", [C, HC], F32R, kind="ExternalInput").ap()
    wk_d = nc.dram_tensor("wk", [C, HC], F32R, kind="ExternalInput").ap()
    wv_d = nc.dram_tensor("wv", [C, HC], F32R, kind="ExternalInput").ap()
    bq_d = nc.dram_tensor("bq", [HC], F32, kind="ExternalInput").ap()
    bk_d = nc.dram_tensor("bk", [HC], F32, kind="ExternalInput").ap()
    wo_d = nc.dram_tensor("wo", [HC, C], F32R, kind="ExternalInput").ap()
    y_d = nc.dram_tensor("y", [T, C], F32, kind="ExternalOutput").ap()

    with tile.TileContext(nc) as tc:
        with (
            tc.tile_pool(name="const", bufs=1) as const,
            tc.tile_pool(name="big", bufs=1) as big,
            tc.tile_pool(name="ysb", bufs=3) as ysb,
            tc.tile_pool(name="stats", bufs=24) as stats,
            tc.tile_pool(name="ps_s", bufs=4, space="PSUM") as ps_s,
            tc.tile_pool(name="ps_t", bufs=2, space="PSUM") as ps_t,
            tc.tile_pool(name="ps_o", bufs=2, space="PSUM") as ps_o,
        ):
            # ---- input DMAs: wq/x(tg0) interleaved per c-chunk so the first
            # projection matmul starts as early as possible ----
            ins_pool = tc.tile_pool(name="ins", bufs=1)
            ins = ins_pool.__enter__()
            wq = ins.tile([P, CG, HC], F32R)
            xT = ins.tile([P, CG, T], F32R)
            wqr = wq_d.rearrange("(o p) n -> p o n", p=P)
            xTr = xT_d.rearrange("(o p) t -> p o t", p=P)
            for c in range(CG):
                nc.sync.dma_start(wq[:, c, :], wqr[:, c, :])
                nc.sync.dma_start(xT[:, c, ts(0, 512)], xTr[:, c, ts(0, 512)])
            wk = ins.tile([P, CG, HC], F32R)
            nc.sync.dma_start(wk, wk_d.rearrange("(o p) n -> p o n", p=P))
            bqT = const.tile([1, HC], F32R)
            nc.gpsimd.dma_start(bqT, bq_d.rearrange("(o n) -> o n", o=1))
            bkT = const.tile([1, HC], F32R)
            nc.gpsimd.dma_start(bkT, bk_d.rearrange("(o n) -> o n", o=1))
            onesr = const.tile([1, 512], F32R)
            # (x*0)+1 from any f32r source row: memset can't target a
            # single-partition f32r tile (ISA check)
            nc.vector.tensor_scalar(
                onesr, xT[0:1, 0, ts(0, 512)], 0.0, 1.0, OP.mult, OP.add
            )
            wv = ins.tile([P, CG, HC], F32R)
            nc.sync.dma_start(wv, wv_d.rearrange("(o p) n -> p o n", p=P))
            for tg in range(1, NG):
                for c2 in range(0, CG, 2):
                    nc.sync.dma_start(
                        xT[:, c2 : c2 + 2, ts(tg, 512)],
                        xTr[:, c2 : c2 + 2, ts(tg, 512)],
                    )
            wo = const.tile([P, HPAIRS, C], F32R)
            nc.sync.dma_start(wo, wo_d.rearrange("(o p) n -> p o n", p=P))

            ident = const.tile([P, P], BF16)
            make_identity(nc, ident)
            # cmask[:, :128] lower-triangular 0/-1e30, cmask[:, 128:256] all -1e30
            cmask = const.tile([P, 256], BF16)
            nc.gpsimd.memset(cmask, 0.0)
            nc.gpsimd.affine_select(
                out=cmask,
                in_=cmask,
                compare_op=OP.is_ge,
                fill=NEG,
                base=0,
                pattern=[[-1, 256]],
                channel_multiplier=1,
            )

            # ---- persistent intermediates ----
            QT = big.tile([P, HPAIRS, T], F32R)   # rows: head-pair's 2 heads x 64, scaled by 8, +bias
            KT = big.tile([P, HPAIRS, T], F32R)
            VS = big.tile([P, TT, HC], BF16)      # V rows: tokens, cols: 4 heads x 64
            OT = big.tile([P, HPAIRS, T], F32R)   # context^T rows: channels
            if "pv" in ablate or "attn" in ablate:
                nc.vector.memset(OT, 0.0)

            # ---- projection subchunk emitters (q/k/v psum shares the score
            # ring: both are [128,512]-class fp32 tiles with short lives) ----
            def emit_proj_q(hp, tg):
                q_ps = ps_s.tile([P, 512], F32, tag="S", name="q_ps")
                for c in range(CG):
                    nc.tensor.matmul(
                        q_ps,
                        wq[:, c, ts(hp, P)],
                        xT[:, c, ts(tg, 512)],
                        start=(c == 0),
                        stop=False,
                    )
                # bias add as a rank-1 accumulate: q_ps += bq ox ones
                nc.tensor.matmul(
                    q_ps,
                    bqT[:, ts(hp, P)],
                    onesr,
                    start=False,
                    stop=True,
                    skip_group_check=True,
                )
                # QT = psum * 8   (fold sqrt(D) score scale into Q)
                nc.scalar.activation(
                    QT[:, hp, ts(tg, 512)], q_ps, ACTF.Copy, bias=0.0, scale=8.0
                )

            def emit_proj_k(hp, tg):
                k_ps = ps_s.tile([P, 512], F32, tag="S", name="k_ps")
                for c in range(CG):
                    nc.tensor.matmul(
                        k_ps,
                        wk[:, c, ts(hp, P)],
                        xT[:, c, ts(tg, 512)],
                        start=(c == 0),
                        stop=False,
                    )
                nc.tensor.matmul(
                    k_ps,
                    bkT[:, ts(hp, P)],
                    onesr,
                    start=False,
                    stop=True,
                    skip_group_check=True,
                )
                nc.scalar.copy(KT[:, hp, ts(tg, 512)], k_ps)

            def emit_proj_v(tg, half):
                for tt in range(4 * tg + 2 * half, 4 * tg + 2 * half + 2):
                    v_ps = ps_s.tile([P, HC], F32, tag="S", name="v_ps")
                    for c in range(CG):
                        nc.tensor.matmul(
                            v_ps,
                            xT[:, c, ts(tt, P)],
                            wv[:, c, :],
                            start=(c == 0),
                            stop=(c == CG - 1),
                        )
                    nc.scalar.copy(VS[:, tt, :], v_ps)

            rot = [0]  # pt_sb copy engine rotation
            arot = [0]  # accum-on-Pool rotation

            # ---------- phase A emitters ----------
            def emit_scores(st, qc):
                """Scores + causal mask + per-part row-max + exp for one q-tile."""
                hp, h, g = st["hp"], st["h"], st["g"]
                hrow = 64 * h
                qt = 4 * g + qc
                L = 512 * g + WLAST[qc]
                np_ = g + 1
                p_t = pp.tile([P, T], BF16, tag="P", name=f"p_{hp}_{h}_{g}_{qc}")
                mparts = stats.tile([P, 4], F32, tag="mp")
                sparts = stats.tile([P, 4], F32, tag="sp")
                for i in range(np_):
                    w = PART if i < np_ - 1 else WLAST[qc]
                    diag = i == np_ - 1
                    s_ps = ps_s.tile([P, PART], F32, tag="S")
                    nc.tensor.matmul(
                        s_ps[:, :w],
                        QT[hrow : hrow + 64, hp, ts(qt, P)],
                        KT[hrow : hrow + 64, hp, ds(PART * i, w)],
                        start=True,
                        stop=not diag,
                    )
                    if diag:
                        # causal mask on the diagonal 128 (+128 pad for qc=0)
                        # added on the PE: s_ps += ident.T @ cmask == cmask
                        mw = 256 if qc == 0 else 128
                        dof = 128 * qt - PART * i
                        nc.tensor.matmul(
                            s_ps[:, dof : dof + mw],
                            ident,
                            cmask[:, :mw],
                            start=False,
                            stop=True,
                            skip_group_check=True,
                        )
                    # negated per-part row max -> exp bias directly
                    nc.vector.reduce_max(
                        mparts[:, i : i + 1], s_ps[:, :w],
                        axis=AX.X, negate=True,
                    )
                    nc.scalar.activation(
                        p_t[:, ds(PART * i, w)], s_ps[:, :w], ACTF.Exp,
                        bias=mparts[:, i : i + 1], scale=1.0,
                        accum_out=sparts[:, i : i + 1],
                    )
                st["p_tiles"][qc] = p_t
                st["stats"][qc] = (mparts, sparts, np_)

            def emit_combine(st, qc):
                """Renorm factors f_i = exp(m_i - m)/s -> diag tiles."""
                mparts, sparts, np_ = st["stats"][qc]
                dgl = []
                if np_ == 1:
                    r = stats.tile([P, 1], F32, tag="r")
                    nc.vector.reciprocal(r, sparts[:, :1])
                    dg0 = dgp.tile([P, P], BF16, tag="dg")
                    nc.gpsimd.tensor_scalar(dg0, ident, r, None, OP.mult)
                    dgl.append(dg0)
                else:
                    negm = stats.tile([P, 1], F32, tag="negm")
                    nc.vector.tensor_reduce(
                        negm, mparts[:, :np_], axis=AX.X, op=OP.min
                    )
                    e = stats.tile([P, 4], F32, tag="e")
                    nc.scalar.activation(
                        e[:, :np_], mparts[:, :np_], ACTF.Exp,
                        bias=negm, scale=-1.0,
                    )
                    z = stats.tile([P, 4], F32, tag="z")
                    nc.vector.tensor_tensor(
                        z[:, :np_], sparts[:, :np_], e[:, :np_], OP.mult
                    )
                    s = stats.tile([P, 1], F32, tag="s")
                    nc.vector.reduce_sum(s, z[:, :np_], axis=AX.X)
                    r = stats.tile([P, 1], F32, tag="r")
                    nc.vector.reciprocal(r, s)
                    for i in range(np_):
                        dg = dgp.tile([P, P], BF16, tag="dg")
                        # dg = (ident * e_i) * r  == ident * f_i
                        nc.gpsimd.tensor_scalar(
                            dg, ident, e[:, i : i + 1], r, OP.mult, OP.mult
                        )
                        dgl.append(dg)
                st["dgs"][qc] = dgl

            # ---------- phase B emitters ----------
            def emit_pv(st, k0, k1):
                """Transpose P k-tiles (scaled by diag(f)) and accumulate P^T@V.
                The P^T@V matmul lags one k-tile behind its PSUM->SBUF copy so
                the in-order PE never waits on the copy engine."""
                if "pv" in ablate:
                    return
                hp, h, g = st["hp"], st["h"], st["g"]
                hcol = (2 * hp + h) * 64
                nks = 4 * g + 4

                def emit_pv_mm(ks, pt_sb, qstart):
                    nc.tensor.matmul(
                        st["o_ps"][:, qstart * P :],
                        VS[:, ks, hcol : hcol + 64],
                        pt_sb[:, qstart * P :],
                        start=(ks == 0),
                        stop=(ks == nks - 1),
                        skip_group_check=True,
                    )

                for ks in range(k0, k1):
                    if ks == 0:
                        st["o_ps"] = ps_o.tile([64, 512], F32, tag="O", name="o_ps")
                    ip = ks * P // PART
                    lsd = ks - 4 * g
                    if lsd < 2:
                        qstart = 0
                    elif lsd == 2:
                        qstart = 2
                    else:
                        qstart = 3
                    pt_ps = ps_t.tile([P, 512], BF16, tag="pT")
                    for qc in range(qstart, 4):
                        nc.tensor.matmul(
                            pt_ps[:, ts(qc, P)],
                            st["p_tiles"][qc][:, ts(ks, P)],
                            st["dgs"][qc][ip],
                            is_transpose=True,
                            skip_group_check=True,
                        )
                    pt_sb = pts.tile([P, 512], BF16, tag="pTs")
                    m = rot[0] % 16
                    rot[0] += 1
                    if m < 11 or projq:
                        nc.vector.tensor_copy(
                            pt_sb[:, qstart * P :], pt_ps[:, qstart * P :]
                        )
                    else:
                        nc.scalar.copy(
                            pt_sb[:, qstart * P :], pt_ps[:, qstart * P :]
                        )
                    st["pv_pending"].append((ks, pt_sb, qstart))
                    if len(st["pv_pending"]) > 2:
                        emit_pv_mm(*st["pv_pending"].pop(0))
                if k1 == nks:
                    while st["pv_pending"]:
                        emit_pv_mm(*st["pv_pending"].pop(0))

            def emit_tail(st):
                """OT writeback; output projection after the last head of a
                q-group (overlaps later attention)."""
                if "pv" in ablate:
                    return
                hp, h, g = st["hp"], st["h"], st["g"]
                hrow = 64 * h
                nc.vector.tensor_copy(
                    OT[hrow : hrow + 64, hp, ts(g, 512)], st["o_ps"]
                )
                if hp == 1 and h == 1:
                    for tt in range(4 * g, 4 * g + 4):
                        for n in range(2):
                            y_ps = ps_o.tile([P, 512], F32, tag="O")
                            for hpp in range(HPAIRS):
                                nc.tensor.matmul(
                                    y_ps,
                                    OT[:, hpp, ts(tt, P)],
                                    wo[:, hpp, ts(n, 512)],
                                    start=(hpp == 0),
                                    stop=(hpp == HPAIRS - 1),
                                )
                            y_sb = ysb.tile([P, 512], F32, tag="y")
                            nc.scalar.copy(y_sb, y_ps)
                            nc.sync.dma_start(
                                y_d[ts(tt, P), ts(n, 512)], y_sb
                            )

            # ---- projection phase (V interleaved with hp0 so attention
            # inputs are ready earliest) ----
            for hp in range(HPAIRS):
                for tg in range(NG):
                    emit_proj_q(hp, tg)
                    emit_proj_k(hp, tg)
                    if hp == 0:
                        emit_proj_v(tg, 0)
                        emit_proj_v(tg, 1)
            ins_pool.__exit__(None, None, None)
            pp_pool = tc.tile_pool(name="pp", bufs=13)
            pp = pp_pool.__enter__()
            pts_pool = tc.tile_pool(name="pts", bufs=8)
            pts = pts_pool.__enter__()
            projq = []

            # ---- software-pipelined attention loop: weave phase B of
            # iteration n-1 between the score tiles of iteration n ----
            its = [
                (hp, h, g)
                for hp in range(HPAIRS if "attn" not in ablate else 0)
                for h in range(2)
                for g in range(NG)
            ]
            if its:
                # last head processes its groups largest-first so the
                # pipeline drain tail is the smallest group + outproj
                its[-NG:] = [(1, 1, g) for g in range(NG - 1, -1, -1)]
            prev = None
            for idx, (hp, h, g) in enumerate(its):
                st = {"hp": hp, "h": h, "g": g, "p_tiles": {}, "dgs": {},
                      "stats": {}, "o_ps": None, "pv_pending": []}
                nks_prev = (4 * prev["g"] + 4) if prev is not None else 0
                bounds = [nks_prev * j // 4 for j in range(5)]
                for qc in range(4):
                    if prev is not None:
                        emit_pv(prev, bounds[qc], bounds[qc + 1])
                    emit_scores(st, qc)
                    if qc > 0:
                        emit_combine(st, qc - 1)
                if prev is not None:
                    emit_tail(prev)
                emit_combine(st, 3)
                prev = st
            if prev is not None:
                emit_pv(prev, 0, 4 * prev["g"] + 4)
                emit_tail(prev)
            pts_pool.__exit__(None, None, None)
            pp_pool.__exit__(None, None, None)

    nc.compile()
    return nc


def kernel(x, w_qkv, b_qkv, b_out, w_out=None, **kw):
    # tolerate arbitrary kwarg order; reference signature is
    # (x, w_qkv, b_qkv, w_out, b_out)
    if w_out is None:
        w_out = kw.pop("w_out")
    global LAST_RESULT
    x = np.asarray(x, dtype=np.float32)
    w_qkv = np.asarray(w_qkv, dtype=np.float32)
    b_qkv = np.asarray(b_qkv, dtype=np.float32)
    w_out = np.asarray(w_out, dtype=np.float32)
    b_out = np.asarray(b_out, dtype=np.float32)

    if "nc" not in _CACHE:
        _CACHE["nc"] = _build()
    nc = _CACHE["nc"]

    xTs = [np.ascontiguousarray(x[b].T) for b in range(B)]
    in_maps = []
    for c in range(8):
        b = c // 4
        k4 = c % 4
        cols = slice(HC * k4, HC * k4 + HC)
        in_maps.append(
            {
                "xT": xTs[b],
                "wq": np.ascontiguousarray(w_qkv[:, cols]),
                "wk": np.ascontiguousarray(w_qkv[:, C + cols.start : C + cols.stop]),
                "wv": np.ascontiguousarray(
                    w_qkv[:, 2 * C + cols.start : 2 * C + cols.stop]
                ),
                "bq": np.ascontiguousarray(b_qkv[cols]),
                "bk": np.ascontiguousarray(b_qkv[C + cols.start : C + cols.stop]),
                "wo": np.ascontiguousarray(w_out[cols, :]),
            }
        )

    res = run_bass_kernel_spmd(nc, in_maps, core_ids=list(range(8)))
    LAST_RESULT = res

    y = np.zeros((B, T, C), dtype=np.float32)
    for c in range(8):
        y[c // 4] += res.results[c]["y"]
    # constant terms: V-bias flows through softmax (weights sum to 1) as a
    # constant row shift, so its contribution is exactly b_v @ w_out; plus b_out.
    b_v = b_qkv[2 * C :]
    y += (b_v @ w_out + b_out).astype(np.float32)
    return y
